# revision 35
# baseline (speedup 1.0000x reference)
"""Distributed multi-head attention layer on 8 TRN2 NeuronCores.

Problem: B=2, S=2048, D=1024, H=16 heads, head_dim=64, fp32.

Sharding: HEAD-parallel, 8 ways. Core c owns heads {2c, 2c+1} (the
128-wide feature slice [128c, 128c+128) of Q/K/V) over BOTH batches,
so K/V projections are computed exactly once fleet-wide (the previous
sequence-parallel kernel recomputed K/V 4x per core = ~109us of extra
PE time). After attention, one 8-core AllToAll (1MB/core, ~10us data
phase measured on this fabric; blocks = 512-row output chunks)
redistributes attended features so core c holds all 16 heads for
global row chunk c, then a local full-width output projection emits
512 rows/core. A dummy AllToAll at kernel start absorbs launch skew
and warms the CC cores (first-collective prep otherwise costs ~11us).

Per core: Q^T/K^T land feature-major [128, 4096] (one 128-slab each);
V is projected feature-major too (512-wide moving dim, full PE
efficiency) and flipped key-major by 32 identity-matmul transposes
into ones-augmented vaug tiles [128 keys, 2*65] - the ones column
yields the softmax denominator inside the attended matmul. Attention
runs as 8 units (batch, 512-query block): per key tile both heads'
scores fill one [128, 1024] PSUM tile via two concurrent 64-row
matmuls and one Exp covers both; attended accumulates in two
8-contiguous PSUM groups per head, combined in f32 on DVE (no
interrupted accumulation groups, no bf16 accumulator).

Scheduling is dominated by the PE p-state rule (full 2.4 GHz only
after ~3us of gap-free execution; idle drops it to 1.2 GHz): the
Scalar engine's 128 Exp tiles (~150us) pace the units, and projection
work for later column chunks rides filler slots between score pairs
sized to keep the PE stream contiguous. Attention starts after only
K(0)+Q(0) (1.25MB of x^T) with K(1..3) as unit-0 fillers ahead of
their score groups. attT halves stage to DRAM as units normalize, and
sacrificial "warm" matmuls occupy the PE during the AllToAll barrier
so the output projection starts at full clock. Empirically rejected:
K/Q bias drains on DVE and a PSUM-based reciprocal broadcast (both
stall PSUM slot release and drop the PE p-state, +30us), and
reciprocal_approx_fast reading a partition-offset AP (returns
garbage - denominators must be copied to a partition-0 tile first),
and gating the PE start on the sync AllToAll (launch skew is 10-60us
and variable; eating it serially at the start plus the dummy's ~13us
overhead is strictly worse than absorbing it at the end barrier under
warm matmuls). Run-to-run variance of the final barrier wait
(~10-30us) dominatesremaining noise.
"""


import sys

sys.path.insert(0, "/opt/trn_rl_repo")

import ml_dtypes
import numpy as np

import concourse.bass as bass
import concourse.tile as tile
from concourse import bacc, mybir
from concourse.bass_utils import run_bass_kernel_spmd

f32 = mybir.dt.float32
bf16 = mybir.dt.bfloat16
ACT = mybir.ActivationFunctionType

B, S, D = 2, 2048, 1024
H, HD = 16, 64
NCORES = 8
GC = B * S  # 4096 global query/key columns
F = D // NCORES  # 128 features (2 heads) per core
NK = D // 128  # 8 contraction tiles
NCC = GC // 512  # 8 column chunks
NKT = GC // 128  # 32 key tiles (global)
R = GC // NCORES  # 512 output rows per core
SCALE = 1.0 / float(np.sqrt(HD))

_COMPILED = {}


def build_nc(use_f32r=True):
    nc = bacc.Bacc("TRN2", target_bir_lowering=False, debug=False, num_devices=8)

    xT = nc.dram_tensor("xT", [D, GC], bf16, kind="ExternalInput")
    Wq = nc.dram_tensor("Wq", [D, F], bf16, kind="ExternalInput")
    Wk = nc.dram_tensor("Wk", [D, F], bf16, kind="ExternalInput")
    Wv = nc.dram_tensor("Wv", [D, F], bf16, kind="ExternalInput")
    Wo = nc.dram_tensor("Wo", [D, D], bf16, kind="ExternalInput")
    bq = nc.dram_tensor("bq", [F, 1], f32, kind="ExternalInput")
    bk = nc.dram_tensor("bk", [F, 1], f32, kind="ExternalInput")
    bv = nc.dram_tensor("bv", [F, 1], f32, kind="ExternalInput")
    bo = nc.dram_tensor("bo", [D, 1], f32, kind="ExternalInput")
    out = nc.dram_tensor("out", [R, D], f32, kind="ExternalOutput")

    def bcast_row(handle, n):
        # [n,1] bias -> [128, n] partition-broadcast DMA source
        return bass.AP(tensor=handle.ap().tensor, offset=0, ap=[[0, 128], [1, n]])

    from contextlib import ExitStack

    with tile.TileContext(nc) as tc, ExitStack() as es:
            wpool = es.enter_context(tc.tile_pool(name="wpool", bufs=24))
            wopool = es.enter_context(tc.tile_pool(name="wopool", bufs=8))
            xbt_pool = es.enter_context(tc.tile_pool(name="xbt", bufs=24))
            k2_pool = es.enter_context(tc.tile_pool(name="k2", bufs=1))
            qt_pool = es.enter_context(tc.tile_pool(name="qt", bufs=1))
            vt_pool = es.enter_context(tc.tile_pool(name="vt", bufs=1))
            vaug_pool = es.enter_context(tc.tile_pool(name="vaug", bufs=NKT))
            attT_pool = es.enter_context(tc.tile_pool(name="attT", bufs=1))
            attF_pool = es.enter_context(tc.tile_pool(name="attF", bufs=8))
            exp_pool = es.enter_context(tc.tile_pool(name="exp_pool", bufs=26))
            attc_pool = es.enter_context(tc.tile_pool(name="attc", bufs=2))
            outp = es.enter_context(tc.tile_pool(name="outp", bufs=2))
            bias_pool = es.enter_context(tc.tile_pool(name="bias", bufs=1))
            small = es.enter_context(tc.tile_pool(name="small", bufs=2))
            dram = es.enter_context(tc.tile_pool(name="dram", bufs=4, space="DRAM"))
            ps_sc = es.enter_context(tc.tile_pool(name="ps_sc", bufs=2, space="PSUM"))
            ps_att = es.enter_context(tc.tile_pool(name="ps_att", bufs=2, space="PSUM"))
            ps_mm = es.enter_context(tc.tile_pool(name="ps_mm", bufs=2, space="PSUM"))
            dma_round = [nc.sync, nc.scalar, nc.gpsimd]

            def load_w(pool, tag, dram_t, k, width):
                t = pool.tile([128, width], bf16, name=f"{tag}{k}", tag=tag)
                dma_round[k % 3].dma_start(
                    out=t, in_=dram_t.ap()[128 * k : 128 * (k + 1), :]
                )
                return t

            wk_sb = [load_w(wpool, "wk", Wk, k, F) for k in range(NK)]
            wq_sb = [load_w(wpool, "wq", Wq, k, F) for k in range(NK)]
            wv_sb = [load_w(wpool, "wv", Wv, k, F) for k in range(NK)]

            bk_sb = bias_pool.tile([128, 1], f32)
            nc.gpsimd.dma_start(out=bk_sb, in_=bk.ap())
            bq_sb = bias_pool.tile([128, 1], f32)
            nc.gpsimd.dma_start(out=bq_sb, in_=bq.ap())
            bv_bc = bias_pool.tile([128, F], f32, name="bv_bc", tag="bc")
            nc.gpsimd.dma_start(out=bv_bc, in_=bcast_row(bv, F))

            # x^T streamed per (column-chunk PAIR, contraction tile):
            # [128, 1024] tiles give 2KB DRAM lines (vs 1KB at 512 cols),
            # materially better per-queue DMA throughput for the 8MB load
            xbt2 = [[None] * NK for _ in range(NCC // 2)]
            for cp in range(NCC // 2):
                for k in range(NK):
                    t = xbt_pool.tile([128, 1024], bf16, name=f"xb{cp}_{k}", tag="xb")
                    dma_round[(cp * NK + k) % 3].dma_start(
                        out=t,
                        in_=xT.ap()[
                            128 * k : 128 * (k + 1), 1024 * cp : 1024 * (cp + 1)
                        ],
                    )
                    xbt2[cp][k] = t

            def xsl(cc, k):
                h = cc % 2
                return xbt2[cc // 2][k][:, 512 * h : 512 * (h + 1)]

            eye_dram = nc.inline_tensor(
                np.eye(128, dtype=ml_dtypes.bfloat16), name="eye128"
            )
            eye_sb = bias_pool.tile([128, 128], bf16, name="eye_sb", tag="eye")
            nc.sync.dma_start(out=eye_sb, in_=eye_dram.ap() if hasattr(eye_dram, "ap") else bass.AP(tensor=eye_dram, offset=0, ap=[[128, 128], [1, 128]]))
            ones64_dram = nc.inline_tensor(
                np.ones((1, 64), np.float32), name="ones64"
            )
            ones64_sb = bias_pool.tile([1, 64], f32, name="ones64_sb", tag="o64")
            nc.sync.dma_start(
                out=ones64_sb,
                in_=bass.AP(tensor=ones64_dram, offset=0, ap=[[0, 1], [1, 64]]),
            )
            ones_dram = nc.inline_tensor(
                np.ones((1, 2), ml_dtypes.bfloat16), name="ones2"
            )
            ones_sb = bias_pool.tile([128, 2], bf16, name="ones_sb", tag="ones")
            nc.gpsimd.dma_start(
                out=ones_sb,
                in_=bass.AP(tensor=ones_dram, offset=0, ap=[[0, 128], [1, 2]]),
            )
            vaug_sb = []
            for kt in range(NKT):
                va = vaug_pool.tile([128, 2 * 65], bf16, name=f"vaug{kt}", tag="va")
                nc.vector.tensor_copy(
                    out=va.rearrange("p (h c) -> p h c", c=65)[:, :, 64:65],
                    in_=ones_sb.rearrange("p (h one) -> p h one", one=1),
                )
                vaug_sb.append(va)

            k2_sb = k2_pool.tile([128, GC], bf16, name="k2_sb", tag="k2")
            vt_sb = vt_pool.tile([128, GC], bf16, name="vt_sb", tag="vt")
            qt_sb = qt_pool.tile([128, GC], bf16, name="qt_sb", tag="qt")
            attT = attT_pool.tile([128, GC], bf16, name="attT", tag="attT")

            # ---------- projection work units ----------
            def k_unit(cc):
                kps = ps_mm.tile([128, 512], f32, tag="mm", name=f"kps{cc}")
                for k in range(NK):
                    nc.tensor.matmul(
                        out=kps[:],
                        lhsT=wk_sb[k][:],
                        rhs=xsl(cc, k),
                        start=(k == 0),
                        stop=(k == NK - 1),
                    )
                nc.scalar.activation(
                    out=k2_sb[:, 512 * cc : 512 * (cc + 1)],
                    in_=kps[:],
                    func=ACT.Identity,
                    bias=bk_sb[:, 0:1],
                    scale=1.0,
                )

            def q_unit(cc):
                qps = ps_mm.tile([128, 512], f32, tag="mm", name=f"qps{cc}")
                for k in range(NK):
                    nc.tensor.matmul(
                        out=qps[:],
                        lhsT=wq_sb[k][:],
                        rhs=xsl(cc, k),
                        start=(k == 0),
                        stop=(k == NK - 1),
                    )
                nc.scalar.activation(
                    out=qt_sb[:, 512 * cc : 512 * (cc + 1)],
                    in_=qps[:],
                    func=ACT.Identity,
                    bias=bq_sb[:, 0:1],
                    scale=1.0,
                )

            def v2_unit(cc):
                # V^T slab: features-major, full 512-free efficiency
                vtp = ps_mm.tile([128, 512], f32, tag="mm", name=f"vtp{cc}")
                for k in range(NK):
                    nc.tensor.matmul(
                        out=vtp[:],
                        lhsT=wv_sb[k][:],
                        rhs=xsl(cc, k),
                        start=(k == 0),
                        stop=(k == NK - 1),
                    )
                nc.vector.tensor_copy(
                    out=vt_sb[:, 512 * cc : 512 * (cc + 1)], in_=vtp[:]
                )

            def v_unit(kt):
                cc, t = kt // 4, kt % 4
                vps = ps_mm.tile([128, F], f32, tag="mm", name=f"vdp{kt}")
                for k in range(NK):
                    nc.tensor.matmul(
                        out=vps[:],
                        lhsT=xsl(cc, k)[:, 128 * t : 128 * (t + 1)],
                        rhs=wv_sb[k][:],
                        start=(k == 0),
                        stop=(k == NK - 1),
                    )
                nc.vector.tensor_add(
                    out=vaug_sb[kt].rearrange("p (h c) -> p h c", c=65)[:, :, 0:64],
                    in0=vps[:].rearrange("p (h c) -> p h c", c=64),
                    in1=bv_bc[:].rearrange("p (h c) -> p h c", c=64),
                )

            def vtr_unit(kt):
                # transpose V^T column block -> key-major vaug via identity mm
                vps = ps_mm.tile([128, F], f32, tag="mm", name=f"vps{kt}")
                nc.tensor.matmul(
                    out=vps[:],
                    lhsT=vt_sb[:, 128 * kt : 128 * (kt + 1)],
                    rhs=eye_sb[:],
                    start=True,
                    stop=True,
                )
                nc.vector.tensor_add(
                    out=vaug_sb[kt].rearrange("p (h c) -> p h c", c=65)[:, :, 0:64],
                    in0=vps[:].rearrange("p (h c) -> p h c", c=64),
                    in1=bv_bc[:].rearrange("p (h c) -> p h c", c=64),
                )

            # warm units: sacrificial matmuls that keep the PE clock ramped
            # while real work is DMA- or Scalar-gated. One complete
            # accumulation group per call, all into one reused psum tile.
            warm_state = {}

            def warm_unit(n=8):
                wp = warm_state.get("tile")
                if wp is None:
                    wp = ps_mm.tile([128, 512], f32, tag="mm", name="warm_ps")
                    warm_state["tile"] = wp
                for i in range(n):
                    nc.tensor.matmul(
                        out=wp[:],
                        lhsT=wq_sb[i % NK][:],
                        rhs=k2_sb[:, 0:512],
                        start=(i == 0),
                        stop=(i == n - 1),
                    )

            def warm_release():
                wp = warm_state.pop("tile", None)
                if wp is not None:
                    scratch = small.tile([1, 1], f32, tag="wsc", name="wscr")
                    nc.vector.tensor_copy(out=scratch, in_=wp[0:1, 0:1])

            def warm_lead(n):
                # pre-prologue warm on weight tiles (arrive within ~2us):
                # keeps the PE clock ramping while x^T streams in
                wp = warm_state.get("tile")
                if wp is None:
                    wp = ps_mm.tile([128, 512], f32, tag="mm", name="warm_ps")
                    warm_state["tile"] = wp
                for i in range(n):
                    nc.tensor.matmul(
                        out=wp[:, 0:128],
                        lhsT=wq_sb[i % NK][:],
                        rhs=wk_sb[(i + 1) % NK][:],
                        start=(i == 0),
                        stop=(i == n - 1),
                    )

            warm_lead(24)
            warm_release()

            # ---------- prologue: just K(0) + Q(0) (1.25MB of xT) ----------
            k_unit(0)
            q_unit(0)

            bo_bc = bias_pool.tile([128, D], f32, name="bo_bc", tag="bc")
            nc.gpsimd.dma_start(out=bo_bc, in_=bcast_row(bo, D))

            # filler units consumed inside attention units (deadline-ordered)
            # per-slot filler schedule: 8 slots per unit (after sc pairs
            # (0,1),(2,3) of each score group). Emission order IS program
            # order - every filler must be emitted before its first reader.
            E = []
            fillers_by_slot = [
                [[(k_unit, (1,))], [(k_unit, (2,)), (k_unit, (3,))],
                 [(v2_unit, (0,))], [(v2_unit, (1,))],
                 [(v2_unit, (2,))], [(v2_unit, (3,))],
                 [(vtr_unit, (kt,)) for kt in range(0, 4)],
                 [(vtr_unit, (kt,)) for kt in range(4, 8)] + [(q_unit, (1,))]],
                [[(vtr_unit, (8,)), (vtr_unit, (9,))],
                 [(vtr_unit, (10,)), (vtr_unit, (11,))],
                 [(vtr_unit, (12,)), (vtr_unit, (13,))],
                 [(vtr_unit, (14,)), (vtr_unit, (15,))],
                 [(k_unit, (4,))], [(k_unit, (5,))], [(q_unit, (2,))], E],
                [[(k_unit, (6,))], [(k_unit, (7,))], [(q_unit, (3,))],
                 E, E, E, E, E],
                [[(q_unit, (4,))], [(v2_unit, (4,))], [(v2_unit, (5,))],
                 [(v2_unit, (6,))], E, E, E, E],
                [[(v2_unit, (7,))],
                 [(vtr_unit, (kt,)) for kt in range(16, 20)],
                 [(vtr_unit, (kt,)) for kt in range(20, 24)],
                 [(vtr_unit, (kt,)) for kt in range(24, 28)],
                 [(vtr_unit, (kt,)) for kt in range(28, 32)],
                 [(q_unit, (5,))], E, E],
                [[(q_unit, (6,))], E, E, E, E, E, E, E],
                [[(q_unit, (7,))], E, E, E, E, E, E, E],
                [E, E, E, E, E, E, E, E],
            ]

            g8 = [list(range(NCORES))]
            a2a_in = dram.tile([NCORES * 128, R], bf16, name="a2a_in")
            a2a_out = dram.tile([NCORES * 128, R], bf16, name="a2a_out")
            dummy_in = dram.tile([8, 32], bf16, name="dummy_in")
            dummy_out = dram.tile([8, 32], bf16, name="dummy_out")
            nc.gpsimd.collective_compute(
                "AllToAll",
                mybir.AluOpType.bypass,
                replica_groups=g8,
                ins=[dummy_in.opt()],
                outs=[dummy_out.opt()],
            )


            # ---------- attention units ----------
            def sps_exp(b, qb, kt):
                sps = ps_sc.tile([128, 1024], f32, tag="sc")
                for o in range(2):
                    nc.tensor.matmul(
                        out=sps[:, 512 * o : 512 * (o + 1)],
                        lhsT=k2_sb[
                            64 * o : 64 * o + 64,
                            2048 * b + 128 * kt : 2048 * b + 128 * (kt + 1),
                        ],
                        rhs=qt_sb[
                            64 * o : 64 * o + 64,
                            2048 * b + 512 * qb : 2048 * b + 512 * (qb + 1),
                        ],
                        start=True,
                        stop=True,
                    )
                ex = exp_pool.tile([128, 1024], bf16, tag="exp")
                nc.scalar.activation(
                    out=ex, in_=sps[:], func=ACT.Exp, bias=0.0, scale=SCALE
                )
                return ex

            exps_of = [None] * 8
            att_ps = {}  # (o, half) -> psum tile of the unit being attended

            def att_chunk(u, ci):
                b = u // 4
                o, half = ci // 2, ci % 2
                tgt = ps_att.tile([65, R], f32, tag="att", name=f"ap{u}_{ci}")
                att_ps[(o, half)] = tgt
                for idx in range(8):
                    kt = 8 * half + idx
                    nc.tensor.matmul(
                        out=tgt[:],
                        lhsT=vaug_sb[16 * b + kt][:, 65 * o : 65 * o + 65],
                        rhs=exps_of[u][kt][:, 512 * o : 512 * (o + 1)],
                        start=(idx == 0),
                        stop=(idx == 7),
                    )

            def combine_normalize(u, o):
                attC = attc_pool.tile([65, R], f32, tag="attc")
                nc.vector.tensor_copy(out=attC, in_=att_ps[(o, 0)][:])
                nc.vector.tensor_add(out=attC, in0=attC, in1=att_ps[(o, 1)][:])
                den_sb = small.tile([1, R], f32, tag="densb")
                nc.vector.tensor_copy(out=den_sb, in_=attC[64:65, :])
                recip = small.tile([1, R], f32, tag="recip")
                nc.vector.reciprocal_approx_fast(out=recip, in_=den_sb)
                den = small.tile([64, R], f32, tag="den")
                nc.gpsimd.partition_broadcast(den, recip, channels=64)
                nc.vector.tensor_mul(
                    out=attT[64 * o : 64 * o + 64, R * u : R * (u + 1)],
                    in0=attC[0:64, :],
                    in1=den,
                )

            def stage(u, o):
                if o == 0:
                    return
                dma_round[u % 3].dma_start(
                    out=a2a_in[128 * u : 128 * (u + 1), :],
                    in_=attT[:, R * u : R * (u + 1)],
                )

            wo_emitted = False
            for u in range(8):
                b, qb = u // 4, u % 4
                slots = fillers_by_slot[u]
                exs = []
                exps_of[u] = exs
                for g in range(4):
                    for j in range(4):
                        exs.append(sps_exp(b, qb, 4 * g + j))
                        if j % 2 == 1:
                            for fn, args in slots[2 * g + j // 2]:
                                fn(*args)
                    if u > 0:
                        att_chunk(u - 1, g)
                        if g == 1:
                            combine_normalize(u - 1, 0)
                            stage(u - 1, 0)
                        elif g == 3:
                            combine_normalize(u - 1, 1)
                            stage(u - 1, 1)
                            if u == 7:
                                # pull unit 7's first attended chunk into the
                                # loop (ps_att slots just freed by the combine)
                                att_chunk(7, 0)
                if u == 3 and not wo_emitted:
                    # Wo rides the vector queue mid-flight; needed after the A2A
                    wo_emitted = True

            wo_sb = []
            for k in range(NK):
                t = wopool.tile([128, D], bf16, name=f"wo{k}", tag="wo")
                nc.sync.dma_start(out=t, in_=Wo.ap()[128 * k : 128 * (k + 1), :])
                wo_sb.append(t)

            # ---------- epilogue: attend unit 7, exchange, project ----------
            att_chunk(7, 1)
            combine_normalize(7, 0)
            stage(7, 0)
            att_chunk(7, 2)
            att_chunk(7, 3)
            combine_normalize(7, 1)
            stage(7, 1)

            nc.gpsimd.collective_compute(
                "AllToAll",
                mybir.AluOpType.bypass,
                replica_groups=g8,
                ins=[a2a_in.opt()],
                outs=[a2a_out.opt()],
            )
            for _ in range(12):
                warm_unit(8)
            warm_release()

            attF = []
            for i in range(NCORES):
                t = attF_pool.tile([128, R], bf16, name=f"attF{i}", tag="attF")
                dma_round[i % 3].dma_start(out=t, in_=a2a_out[128 * i : 128 * (i + 1), :])
                attF.append(t)

            # 4-deep PSUM ring across the three pools (all free post-attention)
            op_pools = [ps_mm, ps_att, ps_sc, ps_mm]
            gi = 0
            for m in range(R // 128):
                for n in range(2):
                    ops = op_pools[gi % 4].tile(
                        [128, 512], f32, name=f"ops{m}_{n}",
                        tag=["mm", "att", "sc", "mm"][gi % 4],
                    )
                    gi += 1
                    for k in range(NK):
                        nc.tensor.matmul(
                            out=ops[:],
                            lhsT=attF[k][:, 128 * m : 128 * (m + 1)],
                            rhs=wo_sb[k][:, 512 * n : 512 * (n + 1)],
                            start=(k == 0),
                            stop=(k == NK - 1),
                        )
                    oev = outp.tile([128, 512], f32, tag="oev")
                    nc.vector.tensor_add(
                        out=oev, in0=ops[:], in1=bo_bc[:, 512 * n : 512 * (n + 1)]
                    )
                    nc.sync.dma_start(
                        out=out.ap()[
                            128 * m : 128 * (m + 1), 512 * n : 512 * (n + 1)
                        ],
                        in_=oev,
                    )
    nc.finalize()
    return nc


def get_nc(use_f32r=True):
    key = use_f32r
    if key not in _COMPILED:
        _COMPILED[key] = build_nc(use_f32r)
    return _COMPILED[key]


def make_in_maps(x, Wq, bq, Wk, bk, Wv, bv, Wo, bo):
    bf = ml_dtypes.bfloat16
    x = np.asarray(x, np.float32).reshape(B * S, D)
    xT_bf = np.ascontiguousarray(x.T.astype(bf))  # [1024, 4096]
    Wo_bf = np.ascontiguousarray(np.asarray(Wo, np.float32).astype(bf))
    bo_col = np.asarray(bo, np.float32).reshape(D, 1)
    Wq = np.asarray(Wq, np.float32)
    Wk = np.asarray(Wk, np.float32)
    Wv = np.asarray(Wv, np.float32)
    bq = np.asarray(bq, np.float32)
    bk = np.asarray(bk, np.float32)
    bv = np.asarray(bv, np.float32)
    in_maps = []
    for c in range(NCORES):
        sl = slice(F * c, F * (c + 1))
        in_maps.append(
            {
                "xT": xT_bf,
                "Wo": Wo_bf,
                "bo": bo_col,
                "Wq": np.ascontiguousarray(Wq[:, sl].astype(bf)),
                "Wk": np.ascontiguousarray(Wk[:, sl].astype(bf)),
                "Wv": np.ascontiguousarray(Wv[:, sl].astype(bf)),
                "bq": np.ascontiguousarray(bq[sl].reshape(F, 1)),
                "bk": np.ascontiguousarray(bk[sl].reshape(F, 1)),
                "bv": np.ascontiguousarray(bv[sl].reshape(F, 1)),
            }
        )
    return in_maps


def gather_out(results):
    outs = [np.asarray(results[c]["out"], np.float32) for c in range(NCORES)]
    return np.concatenate(outs, axis=0).reshape(B, S, D)


def kernel(x, Wq, bq, Wk, bk, Wv, bv, Wo, bo, _use_f32r=True):
    in_maps = make_in_maps(x, Wq, bq, Wk, bk, Wv, bv, Wo, bo)
    nc = get_nc(_use_f32r)
    res = run_bass_kernel_spmd(nc, in_maps, list(range(NCORES)))
    return gather_out(res.results)


# revision 36
# speedup vs baseline: 1.0055x; 1.0055x over previous
"""Distributed multi-head attention layer on 8 TRN2 NeuronCores.

Problem: B=2, S=2048, D=1024, H=16 heads, head_dim=64, fp32.

Sharding: HEAD-parallel, 8 ways. Core c owns heads {2c, 2c+1} (the
128-wide feature slice [128c, 128c+128) of Q/K/V) over BOTH batches,
so K/V projections are computed exactly once fleet-wide (the previous
sequence-parallel kernel recomputed K/V 4x per core = ~109us of extra
PE time). After attention, one 8-core AllToAll (1MB/core, ~10us data
phase measured on this fabric; blocks = 512-row output chunks)
redistributes attended features so core c holds all 16 heads for
global row chunk c, then a local full-width output projection emits
512 rows/core. A dummy AllToAll at kernel start absorbs launch skew
and warms the CC cores (first-collective prep otherwise costs ~11us).

Per core: Q^T/K^T land feature-major [128, 4096] (one 128-slab each);
V is projected feature-major too (512-wide moving dim, full PE
efficiency) and flipped key-major by 32 identity-matmul transposes
into ones-augmented vaug tiles [128 keys, 2*65] - the ones column
yields the softmax denominator inside the attended matmul. Attention
runs as 8 units (batch, 512-query block): per key tile both heads'
scores fill one [128, 1024] PSUM tile via two concurrent 64-row
matmuls and one Exp covers both; attended accumulates in two
8-contiguous PSUM groups per head, combined in f32 on DVE (no
interrupted accumulation groups, no bf16 accumulator).

Scheduling is dominated by the PE p-state rule (full 2.4 GHz only
after ~3us of gap-free execution; idle drops it to 1.2 GHz): the
Scalar engine's 128 Exp tiles (~150us) pace the units, and projection
work for later column chunks rides filler slots between score pairs
sized to keep the PE stream contiguous. Attention starts after only
K(0)+Q(0) (1.25MB of x^T) with K(1..3) as unit-0 fillers ahead of
their score groups. attT halves stage to DRAM as units normalize, and
sacrificial "warm" matmuls occupy the PE during the AllToAll barrier
so the output projection starts at full clock. Empirically rejected:
K/Q bias drains on DVE and a PSUM-based reciprocal broadcast (both
stall PSUM slot release and drop the PE p-state, +30us), and
reciprocal_approx_fast reading a partition-offset AP (returns
garbage - denominators must be copied to a partition-0 tile first),
and gating the PE start on the sync AllToAll (launch skew is 10-60us
and variable; eating it serially at the start plus the dummy's ~13us
overhead is strictly worse than absorbing it at the end barrier under
warm matmuls). Run-to-run variance of the final barrier wait
(~10-30us) dominatesremaining noise.
"""


import sys

sys.path.insert(0, "/opt/trn_rl_repo")

import ml_dtypes
import numpy as np

import concourse.bass as bass
import concourse.tile as tile
from concourse import bacc, mybir
from concourse.bass_utils import run_bass_kernel_spmd

f32 = mybir.dt.float32
bf16 = mybir.dt.bfloat16
ACT = mybir.ActivationFunctionType

B, S, D = 2, 2048, 1024
H, HD = 16, 64
NCORES = 8
GC = B * S  # 4096 global query/key columns
F = D // NCORES  # 128 features (2 heads) per core
NK = D // 128  # 8 contraction tiles
NCC = GC // 512  # 8 column chunks
NKT = GC // 128  # 32 key tiles (global)
R = GC // NCORES  # 512 output rows per core
SCALE = 1.0 / float(np.sqrt(HD))

_COMPILED = {}


def build_nc(use_f32r=True):
    nc = bacc.Bacc("TRN2", target_bir_lowering=False, debug=False, num_devices=8)

    # host pre-tiles x^T so every [128, 1024] SBUF tile is one contiguous
    # 256KB DRAM read (index = cp*NK + k)
    xT = nc.dram_tensor("xT", [(GC // 1024) * NK, 128, 1024], bf16, kind="ExternalInput")
    Wq = nc.dram_tensor("Wq", [D, F], bf16, kind="ExternalInput")
    Wk = nc.dram_tensor("Wk", [D, F], bf16, kind="ExternalInput")
    Wv = nc.dram_tensor("Wv", [D, F], bf16, kind="ExternalInput")
    Wo = nc.dram_tensor("Wo", [D, D], bf16, kind="ExternalInput")
    bq = nc.dram_tensor("bq", [F, 1], f32, kind="ExternalInput")
    bk = nc.dram_tensor("bk", [F, 1], f32, kind="ExternalInput")
    bv = nc.dram_tensor("bv", [F, 1], f32, kind="ExternalInput")
    bo = nc.dram_tensor("bo", [D, 1], f32, kind="ExternalInput")
    out = nc.dram_tensor("out", [R, D], f32, kind="ExternalOutput")

    def bcast_row(handle, n):
        # [n,1] bias -> [128, n] partition-broadcast DMA source
        return bass.AP(tensor=handle.ap().tensor, offset=0, ap=[[0, 128], [1, n]])

    from contextlib import ExitStack

    with tile.TileContext(nc) as tc, ExitStack() as es:
            wpool = es.enter_context(tc.tile_pool(name="wpool", bufs=24))
            wopool = es.enter_context(tc.tile_pool(name="wopool", bufs=8))
            xbt_pool = es.enter_context(tc.tile_pool(name="xbt", bufs=24))
            k2_pool = es.enter_context(tc.tile_pool(name="k2", bufs=1))
            qt_pool = es.enter_context(tc.tile_pool(name="qt", bufs=1))
            vt_pool = es.enter_context(tc.tile_pool(name="vt", bufs=1))
            vaug_pool = es.enter_context(tc.tile_pool(name="vaug", bufs=NKT))
            attT_pool = es.enter_context(tc.tile_pool(name="attT", bufs=1))
            attF_pool = es.enter_context(tc.tile_pool(name="attF", bufs=8))
            exp_pool = es.enter_context(tc.tile_pool(name="exp_pool", bufs=26))
            attc_pool = es.enter_context(tc.tile_pool(name="attc", bufs=2))
            outp = es.enter_context(tc.tile_pool(name="outp", bufs=2))
            bias_pool = es.enter_context(tc.tile_pool(name="bias", bufs=1))
            small = es.enter_context(tc.tile_pool(name="small", bufs=2))
            dram = es.enter_context(tc.tile_pool(name="dram", bufs=4, space="DRAM"))
            ps_sc = es.enter_context(tc.tile_pool(name="ps_sc", bufs=2, space="PSUM"))
            ps_att = es.enter_context(tc.tile_pool(name="ps_att", bufs=2, space="PSUM"))
            ps_mm = es.enter_context(tc.tile_pool(name="ps_mm", bufs=2, space="PSUM"))
            dma_round = [nc.sync, nc.scalar, nc.gpsimd]

            def load_w(pool, tag, dram_t, k, width):
                t = pool.tile([128, width], bf16, name=f"{tag}{k}", tag=tag)
                dma_round[k % 3].dma_start(
                    out=t, in_=dram_t.ap()[128 * k : 128 * (k + 1), :]
                )
                return t

            wk_sb = [load_w(wpool, "wk", Wk, k, F) for k in range(NK)]
            wq_sb = [load_w(wpool, "wq", Wq, k, F) for k in range(NK)]
            wv_sb = [load_w(wpool, "wv", Wv, k, F) for k in range(NK)]

            bk_sb = bias_pool.tile([128, 1], f32)
            nc.gpsimd.dma_start(out=bk_sb, in_=bk.ap())
            bq_sb = bias_pool.tile([128, 1], f32)
            nc.gpsimd.dma_start(out=bq_sb, in_=bq.ap())
            bv_bc = bias_pool.tile([128, F], f32, name="bv_bc", tag="bc")
            nc.gpsimd.dma_start(out=bv_bc, in_=bcast_row(bv, F))

            # x^T streamed per (column-chunk PAIR, contraction tile):
            # [128, 1024] tiles give 2KB DRAM lines (vs 1KB at 512 cols),
            # materially better per-queue DMA throughput for the 8MB load
            xbt2 = [[None] * NK for _ in range(NCC // 2)]
            for cp in range(NCC // 2):
                for k in range(NK):
                    t = xbt_pool.tile([128, 1024], bf16, name=f"xb{cp}_{k}", tag="xb")
                    dma_round[(cp * NK + k) % 3].dma_start(
                        out=t,
                        in_=xT.ap()[cp * NK + k],
                    )
                    xbt2[cp][k] = t

            def xsl(cc, k):
                h = cc % 2
                return xbt2[cc // 2][k][:, 512 * h : 512 * (h + 1)]

            eye_dram = nc.inline_tensor(
                np.eye(128, dtype=ml_dtypes.bfloat16), name="eye128"
            )
            eye_sb = bias_pool.tile([128, 128], bf16, name="eye_sb", tag="eye")
            nc.sync.dma_start(out=eye_sb, in_=eye_dram.ap() if hasattr(eye_dram, "ap") else bass.AP(tensor=eye_dram, offset=0, ap=[[128, 128], [1, 128]]))
            ones64_dram = nc.inline_tensor(
                np.ones((1, 64), np.float32), name="ones64"
            )
            ones64_sb = bias_pool.tile([1, 64], f32, name="ones64_sb", tag="o64")
            nc.sync.dma_start(
                out=ones64_sb,
                in_=bass.AP(tensor=ones64_dram, offset=0, ap=[[0, 1], [1, 64]]),
            )
            ones_dram = nc.inline_tensor(
                np.ones((1, 2), ml_dtypes.bfloat16), name="ones2"
            )
            ones_sb = bias_pool.tile([128, 2], bf16, name="ones_sb", tag="ones")
            nc.gpsimd.dma_start(
                out=ones_sb,
                in_=bass.AP(tensor=ones_dram, offset=0, ap=[[0, 128], [1, 2]]),
            )
            vaug_sb = []
            for kt in range(NKT):
                va = vaug_pool.tile([128, 2 * 65], bf16, name=f"vaug{kt}", tag="va")
                nc.vector.tensor_copy(
                    out=va.rearrange("p (h c) -> p h c", c=65)[:, :, 64:65],
                    in_=ones_sb.rearrange("p (h one) -> p h one", one=1),
                )
                vaug_sb.append(va)

            k2_sb = k2_pool.tile([128, GC], bf16, name="k2_sb", tag="k2")
            vt_sb = vt_pool.tile([128, GC], bf16, name="vt_sb", tag="vt")
            qt_sb = qt_pool.tile([128, GC], bf16, name="qt_sb", tag="qt")
            attT = attT_pool.tile([128, GC], bf16, name="attT", tag="attT")

            # ---------- projection work units ----------
            def k_unit(cc):
                kps = ps_mm.tile([128, 512], f32, tag="mm", name=f"kps{cc}")
                for k in range(NK):
                    nc.tensor.matmul(
                        out=kps[:],
                        lhsT=wk_sb[k][:],
                        rhs=xsl(cc, k),
                        start=(k == 0),
                        stop=(k == NK - 1),
                    )
                nc.scalar.activation(
                    out=k2_sb[:, 512 * cc : 512 * (cc + 1)],
                    in_=kps[:],
                    func=ACT.Identity,
                    bias=bk_sb[:, 0:1],
                    scale=1.0,
                )

            def q_unit(cc):
                qps = ps_mm.tile([128, 512], f32, tag="mm", name=f"qps{cc}")
                for k in range(NK):
                    nc.tensor.matmul(
                        out=qps[:],
                        lhsT=wq_sb[k][:],
                        rhs=xsl(cc, k),
                        start=(k == 0),
                        stop=(k == NK - 1),
                    )
                nc.scalar.activation(
                    out=qt_sb[:, 512 * cc : 512 * (cc + 1)],
                    in_=qps[:],
                    func=ACT.Identity,
                    bias=bq_sb[:, 0:1],
                    scale=1.0,
                )

            def v2_unit(cc):
                # V^T slab: features-major, full 512-free efficiency
                vtp = ps_mm.tile([128, 512], f32, tag="mm", name=f"vtp{cc}")
                for k in range(NK):
                    nc.tensor.matmul(
                        out=vtp[:],
                        lhsT=wv_sb[k][:],
                        rhs=xsl(cc, k),
                        start=(k == 0),
                        stop=(k == NK - 1),
                    )
                nc.vector.tensor_copy(
                    out=vt_sb[:, 512 * cc : 512 * (cc + 1)], in_=vtp[:]
                )

            def v_unit(kt):
                cc, t = kt // 4, kt % 4
                vps = ps_mm.tile([128, F], f32, tag="mm", name=f"vdp{kt}")
                for k in range(NK):
                    nc.tensor.matmul(
                        out=vps[:],
                        lhsT=xsl(cc, k)[:, 128 * t : 128 * (t + 1)],
                        rhs=wv_sb[k][:],
                        start=(k == 0),
                        stop=(k == NK - 1),
                    )
                nc.vector.tensor_add(
                    out=vaug_sb[kt].rearrange("p (h c) -> p h c", c=65)[:, :, 0:64],
                    in0=vps[:].rearrange("p (h c) -> p h c", c=64),
                    in1=bv_bc[:].rearrange("p (h c) -> p h c", c=64),
                )

            def vtr_unit(kt):
                # transpose V^T column block -> key-major vaug via identity mm
                vps = ps_mm.tile([128, F], f32, tag="mm", name=f"vps{kt}")
                nc.tensor.matmul(
                    out=vps[:],
                    lhsT=vt_sb[:, 128 * kt : 128 * (kt + 1)],
                    rhs=eye_sb[:],
                    start=True,
                    stop=True,
                )
                nc.vector.tensor_add(
                    out=vaug_sb[kt].rearrange("p (h c) -> p h c", c=65)[:, :, 0:64],
                    in0=vps[:].rearrange("p (h c) -> p h c", c=64),
                    in1=bv_bc[:].rearrange("p (h c) -> p h c", c=64),
                )

            # warm units: sacrificial matmuls that keep the PE clock ramped
            # while real work is DMA- or Scalar-gated. One complete
            # accumulation group per call, all into one reused psum tile.
            warm_state = {}

            def warm_unit(n=8):
                wp = warm_state.get("tile")
                if wp is None:
                    wp = ps_mm.tile([128, 512], f32, tag="mm", name="warm_ps")
                    warm_state["tile"] = wp
                for i in range(n):
                    nc.tensor.matmul(
                        out=wp[:],
                        lhsT=wq_sb[i % NK][:],
                        rhs=k2_sb[:, 0:512],
                        start=(i == 0),
                        stop=(i == n - 1),
                    )

            def warm_release():
                wp = warm_state.pop("tile", None)
                if wp is not None:
                    scratch = small.tile([1, 1], f32, tag="wsc", name="wscr")
                    nc.vector.tensor_copy(out=scratch, in_=wp[0:1, 0:1])

            def warm_lead(n):
                # pre-prologue warm on weight tiles (arrive within ~2us):
                # keeps the PE clock ramping while x^T streams in
                wp = warm_state.get("tile")
                if wp is None:
                    wp = ps_mm.tile([128, 512], f32, tag="mm", name="warm_ps")
                    warm_state["tile"] = wp
                for i in range(n):
                    nc.tensor.matmul(
                        out=wp[:, 0:128],
                        lhsT=wq_sb[i % NK][:],
                        rhs=wk_sb[(i + 1) % NK][:],
                        start=(i == 0),
                        stop=(i == n - 1),
                    )

            warm_lead(24)
            warm_release()

            # ---------- prologue: just K(0) + Q(0) (1.25MB of xT) ----------
            k_unit(0)
            q_unit(0)

            bo_bc = bias_pool.tile([128, D], f32, name="bo_bc", tag="bc")
            nc.gpsimd.dma_start(out=bo_bc, in_=bcast_row(bo, D))

            # filler units consumed inside attention units (deadline-ordered)
            # per-slot filler schedule: 8 slots per unit (after sc pairs
            # (0,1),(2,3) of each score group). Emission order IS program
            # order - every filler must be emitted before its first reader.
            E = []
            fillers_by_slot = [
                [[(k_unit, (1,))], [(k_unit, (2,)), (k_unit, (3,))],
                 [(v2_unit, (0,))], [(v2_unit, (1,))],
                 [(v2_unit, (2,))], [(v2_unit, (3,))],
                 [(vtr_unit, (kt,)) for kt in range(0, 4)],
                 [(vtr_unit, (kt,)) for kt in range(4, 8)] + [(q_unit, (1,))]],
                [[(vtr_unit, (8,)), (vtr_unit, (9,))],
                 [(vtr_unit, (10,)), (vtr_unit, (11,))],
                 [(vtr_unit, (12,)), (vtr_unit, (13,))],
                 [(vtr_unit, (14,)), (vtr_unit, (15,))],
                 [(k_unit, (4,))], [(k_unit, (5,))], [(q_unit, (2,))], E],
                [[(k_unit, (6,))], [(k_unit, (7,))], [(q_unit, (3,))],
                 E, E, E, E, E],
                [[(q_unit, (4,))], [(v2_unit, (4,))], [(v2_unit, (5,))],
                 [(v2_unit, (6,))], E, E, E, E],
                [[(v2_unit, (7,))],
                 [(vtr_unit, (kt,)) for kt in range(16, 20)],
                 [(vtr_unit, (kt,)) for kt in range(20, 24)],
                 [(vtr_unit, (kt,)) for kt in range(24, 28)],
                 [(vtr_unit, (kt,)) for kt in range(28, 32)],
                 [(q_unit, (5,))], E, E],
                [[(q_unit, (6,))], E, E, E, E, E, E, E],
                [[(q_unit, (7,))], E, E, E, E, E, E, E],
                [E, E, E, E, E, E, E, E],
            ]

            g8 = [list(range(NCORES))]
            a2a_in = dram.tile([NCORES * 128, R], bf16, name="a2a_in")
            a2a_out = dram.tile([NCORES * 128, R], bf16, name="a2a_out")
            dummy_in = dram.tile([8, 32], bf16, name="dummy_in")
            dummy_out = dram.tile([8, 32], bf16, name="dummy_out")
            nc.gpsimd.collective_compute(
                "AllToAll",
                mybir.AluOpType.bypass,
                replica_groups=g8,
                ins=[dummy_in.opt()],
                outs=[dummy_out.opt()],
            )


            # ---------- attention units ----------
            def sps_exp(b, qb, kt):
                sps = ps_sc.tile([128, 1024], f32, tag="sc")
                for o in range(2):
                    nc.tensor.matmul(
                        out=sps[:, 512 * o : 512 * (o + 1)],
                        lhsT=k2_sb[
                            64 * o : 64 * o + 64,
                            2048 * b + 128 * kt : 2048 * b + 128 * (kt + 1),
                        ],
                        rhs=qt_sb[
                            64 * o : 64 * o + 64,
                            2048 * b + 512 * qb : 2048 * b + 512 * (qb + 1),
                        ],
                        start=True,
                        stop=True,
                    )
                ex = exp_pool.tile([128, 1024], bf16, tag="exp")
                nc.scalar.activation(
                    out=ex, in_=sps[:], func=ACT.Exp, bias=0.0, scale=SCALE
                )
                return ex

            exps_of = [None] * 8
            att_ps = {}  # (o, half) -> psum tile of the unit being attended

            def att_chunk(u, ci):
                b = u // 4
                o, half = ci // 2, ci % 2
                tgt = ps_att.tile([65, R], f32, tag="att", name=f"ap{u}_{ci}")
                att_ps[(o, half)] = tgt
                for idx in range(8):
                    kt = 8 * half + idx
                    nc.tensor.matmul(
                        out=tgt[:],
                        lhsT=vaug_sb[16 * b + kt][:, 65 * o : 65 * o + 65],
                        rhs=exps_of[u][kt][:, 512 * o : 512 * (o + 1)],
                        start=(idx == 0),
                        stop=(idx == 7),
                    )

            def combine_normalize(u, o):
                attC = attc_pool.tile([65, R], f32, tag="attc")
                nc.vector.tensor_copy(out=attC, in_=att_ps[(o, 0)][:])
                nc.vector.tensor_add(out=attC, in0=attC, in1=att_ps[(o, 1)][:])
                den_sb = small.tile([1, R], f32, tag="densb")
                nc.vector.tensor_copy(out=den_sb, in_=attC[64:65, :])
                recip = small.tile([1, R], f32, tag="recip")
                nc.vector.reciprocal_approx_fast(out=recip, in_=den_sb)
                den = small.tile([64, R], f32, tag="den")
                nc.gpsimd.partition_broadcast(den, recip, channels=64)
                nc.vector.tensor_mul(
                    out=attT[64 * o : 64 * o + 64, R * u : R * (u + 1)],
                    in0=attC[0:64, :],
                    in1=den,
                )

            def stage(u, o):
                if o == 0:
                    return
                dma_round[u % 3].dma_start(
                    out=a2a_in[128 * u : 128 * (u + 1), :],
                    in_=attT[:, R * u : R * (u + 1)],
                )

            wo_emitted = False
            for u in range(8):
                b, qb = u // 4, u % 4
                slots = fillers_by_slot[u]
                exs = []
                exps_of[u] = exs
                for g in range(4):
                    for j in range(4):
                        exs.append(sps_exp(b, qb, 4 * g + j))
                        if j % 2 == 1:
                            for fn, args in slots[2 * g + j // 2]:
                                fn(*args)
                    if u > 0:
                        att_chunk(u - 1, g)
                        if g == 1:
                            combine_normalize(u - 1, 0)
                            stage(u - 1, 0)
                        elif g == 3:
                            combine_normalize(u - 1, 1)
                            stage(u - 1, 1)
                            if u == 7:
                                # pull unit 7's first attended chunk into the
                                # loop (ps_att slots just freed by the combine)
                                att_chunk(7, 0)
                if u == 3 and not wo_emitted:
                    # Wo rides the vector queue mid-flight; needed after the A2A
                    wo_emitted = True

            wo_sb = []
            for k in range(NK):
                t = wopool.tile([128, D], bf16, name=f"wo{k}", tag="wo")
                nc.sync.dma_start(out=t, in_=Wo.ap()[128 * k : 128 * (k + 1), :])
                wo_sb.append(t)

            # ---------- epilogue: attend unit 7, exchange, project ----------
            att_chunk(7, 1)
            combine_normalize(7, 0)
            stage(7, 0)
            att_chunk(7, 2)
            att_chunk(7, 3)
            combine_normalize(7, 1)
            stage(7, 1)

            nc.gpsimd.collective_compute(
                "AllToAll",
                mybir.AluOpType.bypass,
                replica_groups=g8,
                ins=[a2a_in.opt()],
                outs=[a2a_out.opt()],
            )
            for _ in range(12):
                warm_unit(8)
            warm_release()

            attF = []
            for i in range(NCORES):
                t = attF_pool.tile([128, R], bf16, name=f"attF{i}", tag="attF")
                dma_round[i % 3].dma_start(out=t, in_=a2a_out[128 * i : 128 * (i + 1), :])
                attF.append(t)

            # 4-deep PSUM ring across the three pools (all free post-attention)
            op_pools = [ps_mm, ps_att, ps_sc, ps_mm]
            gi = 0
            for m in range(R // 128):
                for n in range(2):
                    ops = op_pools[gi % 4].tile(
                        [128, 512], f32, name=f"ops{m}_{n}",
                        tag=["mm", "att", "sc", "mm"][gi % 4],
                    )
                    gi += 1
                    for k in range(NK):
                        nc.tensor.matmul(
                            out=ops[:],
                            lhsT=attF[k][:, 128 * m : 128 * (m + 1)],
                            rhs=wo_sb[k][:, 512 * n : 512 * (n + 1)],
                            start=(k == 0),
                            stop=(k == NK - 1),
                        )
                    oev = outp.tile([128, 512], f32, tag="oev")
                    nc.vector.tensor_add(
                        out=oev, in0=ops[:], in1=bo_bc[:, 512 * n : 512 * (n + 1)]
                    )
                    nc.sync.dma_start(
                        out=out.ap()[
                            128 * m : 128 * (m + 1), 512 * n : 512 * (n + 1)
                        ],
                        in_=oev,
                    )
    nc.finalize()
    return nc


def get_nc(use_f32r=True):
    key = use_f32r
    if key not in _COMPILED:
        _COMPILED[key] = build_nc(use_f32r)
    return _COMPILED[key]


def make_in_maps(x, Wq, bq, Wk, bk, Wv, bv, Wo, bo):
    bf = ml_dtypes.bfloat16
    x = np.asarray(x, np.float32).reshape(B * S, D)
    xt = x.T.astype(bf)  # [1024, 4096]
    # pre-tile to [32, 128, 1024]: index = cp*8 + k, each tile contiguous
    xT_bf = np.ascontiguousarray(
        np.stack(
            [
                xt[128 * k : 128 * (k + 1), 1024 * cp : 1024 * (cp + 1)]
                for cp in range(4)
                for k in range(8)
            ]
        )
    )
    Wo_bf = np.ascontiguousarray(np.asarray(Wo, np.float32).astype(bf))
    bo_col = np.asarray(bo, np.float32).reshape(D, 1)
    Wq = np.asarray(Wq, np.float32)
    Wk = np.asarray(Wk, np.float32)
    Wv = np.asarray(Wv, np.float32)
    bq = np.asarray(bq, np.float32)
    bk = np.asarray(bk, np.float32)
    bv = np.asarray(bv, np.float32)
    in_maps = []
    for c in range(NCORES):
        sl = slice(F * c, F * (c + 1))
        in_maps.append(
            {
                "xT": xT_bf,
                "Wo": Wo_bf,
                "bo": bo_col,
                "Wq": np.ascontiguousarray(Wq[:, sl].astype(bf)),
                "Wk": np.ascontiguousarray(Wk[:, sl].astype(bf)),
                "Wv": np.ascontiguousarray(Wv[:, sl].astype(bf)),
                "bq": np.ascontiguousarray(bq[sl].reshape(F, 1)),
                "bk": np.ascontiguousarray(bk[sl].reshape(F, 1)),
                "bv": np.ascontiguousarray(bv[sl].reshape(F, 1)),
            }
        )
    return in_maps


def gather_out(results):
    outs = [np.asarray(results[c]["out"], np.float32) for c in range(NCORES)]
    return np.concatenate(outs, axis=0).reshape(B, S, D)


def kernel(x, Wq, bq, Wk, bk, Wv, bv, Wo, bo, _use_f32r=True):
    in_maps = make_in_maps(x, Wq, bq, Wk, bk, Wv, bv, Wo, bo)
    nc = get_nc(_use_f32r)
    res = run_bass_kernel_spmd(nc, in_maps, list(range(NCORES)))
    return gather_out(res.results)


# revision 37
# speedup vs baseline: 1.0208x; 1.0152x over previous
"""Distributed multi-head attention layer on 8 TRN2 NeuronCores.

Problem: B=2, S=2048, D=1024, H=16 heads, head_dim=64, fp32.

Sharding: HEAD-parallel, 8 ways. Core c owns heads {2c, 2c+1} (the
128-wide feature slice [128c, 128c+128) of Q/K/V) over BOTH batches,
so K/V projections are computed exactly once fleet-wide (the previous
sequence-parallel kernel recomputed K/V 4x per core = ~109us of extra
PE time). After attention, one 8-core AllToAll (1MB/core, ~10us data
phase measured on this fabric; blocks = 512-row output chunks)
redistributes attended features so core c holds all 16 heads for
global row chunk c, then a local full-width output projection emits
512 rows/core. A dummy AllToAll at kernel start absorbs launch skew
and warms the CC cores (first-collective prep otherwise costs ~11us).

Per core: Q^T/K^T land feature-major [128, 4096] (one 128-slab each);
V is projected feature-major too (512-wide moving dim, full PE
efficiency) and flipped key-major by 32 identity-matmul transposes
into ones-augmented vaug tiles [128 keys, 2*65] - the ones column
yields the softmax denominator inside the attended matmul. Attention
runs as 8 units (batch, 512-query block): per key tile both heads'
scores fill one [128, 1024] PSUM tile via two concurrent 64-row
matmuls and one Exp covers both; attended accumulates in two
8-contiguous PSUM groups per head, combined in f32 on DVE (no
interrupted accumulation groups, no bf16 accumulator).

Scheduling is dominated by the PE p-state rule (full 2.4 GHz only
after ~3us of gap-free execution; idle drops it to 1.2 GHz): the
Scalar engine's 128 Exp tiles (~150us) pace the units, and projection
work for later column chunks rides filler slots between score pairs
sized to keep the PE stream contiguous. Attention starts after only
K(0)+Q(0) (1.25MB of x^T) with K(1..3) as unit-0 fillers ahead of
their score groups. attT halves stage to DRAM as units normalize, and
sacrificial "warm" matmuls occupy the PE during the AllToAll barrier
so the output projection starts at full clock. Empirically rejected:
K/Q bias drains on DVE and a PSUM-based reciprocal broadcast (both
stall PSUM slot release and drop the PE p-state, +30us), and
reciprocal_approx_fast reading a partition-offset AP (returns
garbage - denominators must be copied to a partition-0 tile first),
and gating the PE start on the sync AllToAll (launch skew is 10-60us
and variable; eating it serially at the start plus the dummy's ~13us
overhead is strictly worse than absorbing it at the end barrier under
warm matmuls). Run-to-run variance of the final barrier wait
(~10-30us) dominatesremaining noise.
"""


import sys

sys.path.insert(0, "/opt/trn_rl_repo")

import ml_dtypes
import numpy as np

import concourse.bass as bass
import concourse.tile as tile
from concourse import bacc, mybir
from concourse.bass_utils import run_bass_kernel_spmd

f32 = mybir.dt.float32
bf16 = mybir.dt.bfloat16
ACT = mybir.ActivationFunctionType

B, S, D = 2, 2048, 1024
H, HD = 16, 64
NCORES = 8
GC = B * S  # 4096 global query/key columns
F = D // NCORES  # 128 features (2 heads) per core
NK = D // 128  # 8 contraction tiles
NCC = GC // 512  # 8 column chunks
NKT = GC // 128  # 32 key tiles (global)
R = GC // NCORES  # 512 output rows per core
SCALE = 1.0 / float(np.sqrt(HD))

_COMPILED = {}


def build_nc(use_f32r=True):
    nc = bacc.Bacc("TRN2", target_bir_lowering=False, debug=False, num_devices=8)

    # host pre-tiles x^T so every [128, 1024] SBUF tile is one contiguous
    # 256KB DRAM read (index = cp*NK + k)
    xT = nc.dram_tensor("xT", [(GC // 1024) * NK, 128, 1024], bf16, kind="ExternalInput")
    Wq = nc.dram_tensor("Wq", [D, F], bf16, kind="ExternalInput")
    Wk = nc.dram_tensor("Wk", [D, F], bf16, kind="ExternalInput")
    Wv = nc.dram_tensor("Wv", [D, F], bf16, kind="ExternalInput")
    Wo = nc.dram_tensor("Wo", [D, D], bf16, kind="ExternalInput")
    bq = nc.dram_tensor("bq", [F, 1], f32, kind="ExternalInput")
    bk = nc.dram_tensor("bk", [F, 1], f32, kind="ExternalInput")
    bv = nc.dram_tensor("bv", [F, 1], f32, kind="ExternalInput")
    bo = nc.dram_tensor("bo", [D, 1], f32, kind="ExternalInput")
    out = nc.dram_tensor("out", [R, D], f32, kind="ExternalOutput")

    def bcast_row(handle, n):
        # [n,1] bias -> [128, n] partition-broadcast DMA source
        return bass.AP(tensor=handle.ap().tensor, offset=0, ap=[[0, 128], [1, n]])

    from contextlib import ExitStack

    with tile.TileContext(nc) as tc, ExitStack() as es:
            wpool = es.enter_context(tc.tile_pool(name="wpool", bufs=24))
            wopool = es.enter_context(tc.tile_pool(name="wopool", bufs=8))
            xbt_pool = es.enter_context(tc.tile_pool(name="xbt", bufs=24))
            k2_pool = es.enter_context(tc.tile_pool(name="k2", bufs=1))
            qt_pool = es.enter_context(tc.tile_pool(name="qt", bufs=1))
            vt_pool = es.enter_context(tc.tile_pool(name="vt", bufs=1))
            vaug_pool = es.enter_context(tc.tile_pool(name="vaug", bufs=NKT))
            attT_pool = es.enter_context(tc.tile_pool(name="attT", bufs=1))
            attF_pool = es.enter_context(tc.tile_pool(name="attF", bufs=8))
            exp_pool = es.enter_context(tc.tile_pool(name="exp_pool", bufs=26))
            attc_pool = es.enter_context(tc.tile_pool(name="attc", bufs=2))
            outp = es.enter_context(tc.tile_pool(name="outp", bufs=2))
            bias_pool = es.enter_context(tc.tile_pool(name="bias", bufs=1))
            small = es.enter_context(tc.tile_pool(name="small", bufs=2))
            dram = es.enter_context(tc.tile_pool(name="dram", bufs=4, space="DRAM"))
            ps_sc = es.enter_context(tc.tile_pool(name="ps_sc", bufs=2, space="PSUM"))
            ps_att = es.enter_context(tc.tile_pool(name="ps_att", bufs=2, space="PSUM"))
            ps_mm = es.enter_context(tc.tile_pool(name="ps_mm", bufs=2, space="PSUM"))
            dma_round = [nc.sync, nc.scalar, nc.gpsimd]

            def load_w(pool, tag, dram_t, k, width):
                t = pool.tile([128, width], bf16, name=f"{tag}{k}", tag=tag)
                dma_round[k % 3].dma_start(
                    out=t, in_=dram_t.ap()[128 * k : 128 * (k + 1), :]
                )
                return t

            wk_sb = [load_w(wpool, "wk", Wk, k, F) for k in range(NK)]
            wq_sb = [load_w(wpool, "wq", Wq, k, F) for k in range(NK)]
            wv_sb = [load_w(wpool, "wv", Wv, k, F) for k in range(NK)]

            bk_sb = bias_pool.tile([128, 1], f32)
            nc.gpsimd.dma_start(out=bk_sb, in_=bk.ap())
            bq_sb = bias_pool.tile([128, 1], f32)
            nc.gpsimd.dma_start(out=bq_sb, in_=bq.ap())
            bv_bc = bias_pool.tile([128, F], f32, name="bv_bc", tag="bc")
            nc.gpsimd.dma_start(out=bv_bc, in_=bcast_row(bv, F))

            # x^T streamed per (column-chunk PAIR, contraction tile):
            # [128, 1024] tiles give 2KB DRAM lines (vs 1KB at 512 cols),
            # materially better per-queue DMA throughput for the 8MB load
            xbt2 = [[None] * NK for _ in range(NCC // 2)]
            for cp in range(NCC // 2):
                for k in range(NK):
                    t = xbt_pool.tile([128, 1024], bf16, name=f"xb{cp}_{k}", tag="xb")
                    dma_round[(cp * NK + k) % 3].dma_start(
                        out=t,
                        in_=xT.ap()[cp * NK + k],
                    )
                    xbt2[cp][k] = t

            def xsl(cc, k):
                h = cc % 2
                return xbt2[cc // 2][k][:, 512 * h : 512 * (h + 1)]

            eye_dram = nc.inline_tensor(
                np.eye(128, dtype=ml_dtypes.bfloat16), name="eye128"
            )
            eye_sb = bias_pool.tile([128, 128], bf16, name="eye_sb", tag="eye")
            nc.sync.dma_start(out=eye_sb, in_=eye_dram.ap() if hasattr(eye_dram, "ap") else bass.AP(tensor=eye_dram, offset=0, ap=[[128, 128], [1, 128]]))
            ones64_dram = nc.inline_tensor(
                np.ones((1, 64), np.float32), name="ones64"
            )
            ones64_sb = bias_pool.tile([1, 64], f32, name="ones64_sb", tag="o64")
            nc.sync.dma_start(
                out=ones64_sb,
                in_=bass.AP(tensor=ones64_dram, offset=0, ap=[[0, 1], [1, 64]]),
            )
            ones_dram = nc.inline_tensor(
                np.ones((1, 2), ml_dtypes.bfloat16), name="ones2"
            )
            ones_sb = bias_pool.tile([128, 2], bf16, name="ones_sb", tag="ones")
            nc.gpsimd.dma_start(
                out=ones_sb,
                in_=bass.AP(tensor=ones_dram, offset=0, ap=[[0, 128], [1, 2]]),
            )
            vaug_sb = []
            for kt in range(NKT):
                va = vaug_pool.tile([128, 2 * 65], bf16, name=f"vaug{kt}", tag="va")
                nc.vector.tensor_copy(
                    out=va.rearrange("p (h c) -> p h c", c=65)[:, :, 64:65],
                    in_=ones_sb.rearrange("p (h one) -> p h one", one=1),
                )
                vaug_sb.append(va)

            k2_sb = k2_pool.tile([128, GC], bf16, name="k2_sb", tag="k2")
            vt_sb = vt_pool.tile([128, GC], bf16, name="vt_sb", tag="vt")
            qt_sb = qt_pool.tile([128, GC], bf16, name="qt_sb", tag="qt")
            attT = attT_pool.tile([128, GC], bf16, name="attT", tag="attT")

            # ---------- projection work units ----------
            def k_unit(cc):
                kps = ps_mm.tile([128, 512], f32, tag="mm", name=f"kps{cc}")
                for k in range(NK):
                    nc.tensor.matmul(
                        out=kps[:],
                        lhsT=wk_sb[k][:],
                        rhs=xsl(cc, k),
                        start=(k == 0),
                        stop=(k == NK - 1),
                    )
                nc.scalar.activation(
                    out=k2_sb[:, 512 * cc : 512 * (cc + 1)],
                    in_=kps[:],
                    func=ACT.Identity,
                    bias=bk_sb[:, 0:1],
                    scale=1.0,
                )

            def q_unit(cc):
                qps = ps_mm.tile([128, 512], f32, tag="mm", name=f"qps{cc}")
                for k in range(NK):
                    nc.tensor.matmul(
                        out=qps[:],
                        lhsT=wq_sb[k][:],
                        rhs=xsl(cc, k),
                        start=(k == 0),
                        stop=(k == NK - 1),
                    )
                nc.scalar.activation(
                    out=qt_sb[:, 512 * cc : 512 * (cc + 1)],
                    in_=qps[:],
                    func=ACT.Identity,
                    bias=bq_sb[:, 0:1],
                    scale=1.0,
                )

            def v2_unit(cc):
                # V^T slab: features-major, full 512-free efficiency
                vtp = ps_mm.tile([128, 512], f32, tag="mm", name=f"vtp{cc}")
                for k in range(NK):
                    nc.tensor.matmul(
                        out=vtp[:],
                        lhsT=wv_sb[k][:],
                        rhs=xsl(cc, k),
                        start=(k == 0),
                        stop=(k == NK - 1),
                    )
                nc.vector.tensor_copy(
                    out=vt_sb[:, 512 * cc : 512 * (cc + 1)], in_=vtp[:]
                )

            def v_unit(kt):
                cc, t = kt // 4, kt % 4
                vps = ps_mm.tile([128, F], f32, tag="mm", name=f"vdp{kt}")
                for k in range(NK):
                    nc.tensor.matmul(
                        out=vps[:],
                        lhsT=xsl(cc, k)[:, 128 * t : 128 * (t + 1)],
                        rhs=wv_sb[k][:],
                        start=(k == 0),
                        stop=(k == NK - 1),
                    )
                nc.vector.tensor_add(
                    out=vaug_sb[kt].rearrange("p (h c) -> p h c", c=65)[:, :, 0:64],
                    in0=vps[:].rearrange("p (h c) -> p h c", c=64),
                    in1=bv_bc[:].rearrange("p (h c) -> p h c", c=64),
                )

            def vtr_unit(kt):
                # transpose V^T column block -> key-major vaug via identity mm
                vps = ps_mm.tile([128, F], f32, tag="mm", name=f"vps{kt}")
                nc.tensor.matmul(
                    out=vps[:],
                    lhsT=vt_sb[:, 128 * kt : 128 * (kt + 1)],
                    rhs=eye_sb[:],
                    start=True,
                    stop=True,
                )
                nc.vector.tensor_add(
                    out=vaug_sb[kt].rearrange("p (h c) -> p h c", c=65)[:, :, 0:64],
                    in0=vps[:].rearrange("p (h c) -> p h c", c=64),
                    in1=bv_bc[:].rearrange("p (h c) -> p h c", c=64),
                )

            # warm units: sacrificial matmuls that keep the PE clock ramped
            # while real work is DMA- or Scalar-gated. One complete
            # accumulation group per call, all into one reused psum tile.
            warm_state = {}

            def warm_unit(n=8):
                wp = warm_state.get("tile")
                if wp is None:
                    wp = ps_mm.tile([128, 512], f32, tag="mm", name="warm_ps")
                    warm_state["tile"] = wp
                for i in range(n):
                    nc.tensor.matmul(
                        out=wp[:],
                        lhsT=wq_sb[i % NK][:],
                        rhs=k2_sb[:, 0:512],
                        start=(i == 0),
                        stop=(i == n - 1),
                    )

            def warm_release():
                wp = warm_state.pop("tile", None)
                if wp is not None:
                    scratch = small.tile([1, 1], f32, tag="wsc", name="wscr")
                    nc.vector.tensor_copy(out=scratch, in_=wp[0:1, 0:1])

            def warm_lead(n):
                # pre-prologue warm on weight tiles (arrive within ~2us):
                # keeps the PE clock ramping while x^T streams in
                wp = warm_state.get("tile")
                if wp is None:
                    wp = ps_mm.tile([128, 512], f32, tag="mm", name="warm_ps")
                    warm_state["tile"] = wp
                for i in range(n):
                    nc.tensor.matmul(
                        out=wp[:, 0:128],
                        lhsT=wq_sb[i % NK][:],
                        rhs=wk_sb[(i + 1) % NK][:],
                        start=(i == 0),
                        stop=(i == n - 1),
                    )

            warm_lead(24)
            warm_release()

            # ---------- prologue: just K(0) + Q(0) (1.25MB of xT) ----------
            k_unit(0)
            q_unit(0)

            bo_bc = bias_pool.tile([128, D], f32, name="bo_bc", tag="bc")
            nc.gpsimd.dma_start(out=bo_bc, in_=bcast_row(bo, D))

            # filler units consumed inside attention units (deadline-ordered)
            # per-slot filler schedule: 8 slots per unit (after sc pairs
            # (0,1),(2,3) of each score group). Emission order IS program
            # order - every filler must be emitted before its first reader.
            E = []
            fillers_by_slot = [
                [[(k_unit, (1,))], [(k_unit, (2,)), (k_unit, (3,))],
                 [(v2_unit, (0,))], [(v2_unit, (1,))],
                 [(v2_unit, (2,))], [(v2_unit, (3,))],
                 [(vtr_unit, (kt,)) for kt in range(0, 4)],
                 [(vtr_unit, (kt,)) for kt in range(4, 8)] + [(q_unit, (1,))]],
                [[(vtr_unit, (8,)), (vtr_unit, (9,))],
                 [(vtr_unit, (10,)), (vtr_unit, (11,))],
                 [(vtr_unit, (12,)), (vtr_unit, (13,))],
                 [(vtr_unit, (14,)), (vtr_unit, (15,))],
                 [(k_unit, (4,))], [(k_unit, (5,))], [(q_unit, (2,))], E],
                [[(k_unit, (6,))], [(k_unit, (7,))], [(q_unit, (3,))],
                 E, E, E, E, E],
                [[(q_unit, (4,))], [(v2_unit, (4,))], [(v2_unit, (5,))],
                 [(v2_unit, (6,))], E, E, E, E],
                [[(v2_unit, (7,))],
                 [(vtr_unit, (kt,)) for kt in range(16, 20)],
                 [(vtr_unit, (kt,)) for kt in range(20, 24)],
                 [(vtr_unit, (kt,)) for kt in range(24, 28)],
                 [(vtr_unit, (kt,)) for kt in range(28, 32)],
                 [(q_unit, (5,))], E, E],
                [[(q_unit, (6,))], E, E, E, E, E, E, E],
                [[(q_unit, (7,))], E, E, E, E, E, E, E],
                [E, E, E, E, E, E, E, E],
            ]

            g8 = [list(range(NCORES))]
            a2a_in = dram.tile([NCORES * 128, R], bf16, name="a2a_in")
            a2a_out = dram.tile([NCORES * 128, R], bf16, name="a2a_out")
            dummy_in = dram.tile([8, 32], bf16, name="dummy_in")
            dummy_out = dram.tile([8, 32], bf16, name="dummy_out")
            nc.gpsimd.collective_compute(
                "AllToAll",
                mybir.AluOpType.bypass,
                replica_groups=g8,
                ins=[dummy_in.opt()],
                outs=[dummy_out.opt()],
            )


            # ---------- attention units ----------
            def sps_exp(b, qb, kt):
                sps = ps_sc.tile([128, 1024], f32, tag="sc")
                for o in range(2):
                    nc.tensor.matmul(
                        out=sps[:, 512 * o : 512 * (o + 1)],
                        lhsT=k2_sb[
                            64 * o : 64 * o + 64,
                            2048 * b + 128 * kt : 2048 * b + 128 * (kt + 1),
                        ],
                        rhs=qt_sb[
                            64 * o : 64 * o + 64,
                            2048 * b + 512 * qb : 2048 * b + 512 * (qb + 1),
                        ],
                        start=True,
                        stop=True,
                    )
                ex = exp_pool.tile([128, 1024], bf16, tag="exp")
                nc.scalar.activation(
                    out=ex, in_=sps[:], func=ACT.Exp, bias=0.0, scale=SCALE
                )
                return ex

            exps_of = [None] * 8
            att_ps = {}  # (o, half) -> psum tile of the unit being attended

            def att_chunk(u, ci):
                b = u // 4
                o, half = ci // 2, ci % 2
                tgt = ps_att.tile([65, R], f32, tag="att", name=f"ap{u}_{ci}")
                att_ps[(o, half)] = tgt
                for idx in range(8):
                    kt = 8 * half + idx
                    nc.tensor.matmul(
                        out=tgt[:],
                        lhsT=vaug_sb[16 * b + kt][:, 65 * o : 65 * o + 65],
                        rhs=exps_of[u][kt][:, 512 * o : 512 * (o + 1)],
                        start=(idx == 0),
                        stop=(idx == 7),
                    )

            def combine_normalize(u, o):
                attC = attc_pool.tile([65, R], f32, tag="attc")
                nc.vector.tensor_copy(out=attC, in_=att_ps[(o, 0)][:])
                nc.vector.tensor_add(out=attC, in0=attC, in1=att_ps[(o, 1)][:])
                den_sb = small.tile([1, R], f32, tag="densb")
                nc.vector.tensor_copy(out=den_sb, in_=attC[64:65, :])
                recip = small.tile([1, R], f32, tag="recip")
                nc.vector.reciprocal_approx_fast(out=recip, in_=den_sb)
                den = small.tile([64, R], f32, tag="den")
                nc.gpsimd.partition_broadcast(den, recip, channels=64)
                nc.vector.tensor_mul(
                    out=attT[64 * o : 64 * o + 64, R * u : R * (u + 1)],
                    in0=attC[0:64, :],
                    in1=den,
                )

            def stage(u, o):
                if o == 0:
                    return
                dma_round[u % 3].dma_start(
                    out=a2a_in[128 * u : 128 * (u + 1), :],
                    in_=attT[:, R * u : R * (u + 1)],
                )

            wo_emitted = False
            for u in range(8):
                b, qb = u // 4, u % 4
                slots = fillers_by_slot[u]
                exs = []
                exps_of[u] = exs
                for g in range(4):
                    for j in range(4):
                        exs.append(sps_exp(b, qb, 4 * g + j))
                        if j % 2 == 1:
                            for fn, args in slots[2 * g + j // 2]:
                                fn(*args)
                    if u > 0:
                        att_chunk(u - 1, g)
                        if g == 1:
                            combine_normalize(u - 1, 0)
                            stage(u - 1, 0)
                        elif g == 3:
                            combine_normalize(u - 1, 1)
                            stage(u - 1, 1)
                            if u == 7:
                                # pull unit 7's first attended chunk into the
                                # loop (ps_att slots just freed by the combine)
                                att_chunk(7, 0)
                if u == 3 and not wo_emitted:
                    # Wo rides the vector queue mid-flight; needed after the A2A
                    wo_emitted = True

            wo_sb = []
            for k in range(NK):
                t = wopool.tile([128, D], bf16, name=f"wo{k}", tag="wo")
                nc.sync.dma_start(out=t, in_=Wo.ap()[128 * k : 128 * (k + 1), :])
                wo_sb.append(t)

            # ---------- epilogue: attend unit 7, exchange, project ----------
            att_chunk(7, 1)
            combine_normalize(7, 0)
            stage(7, 0)
            att_chunk(7, 2)
            att_chunk(7, 3)
            combine_normalize(7, 1)
            stage(7, 1)

            nc.gpsimd.collective_compute(
                "AllToAll",
                mybir.AluOpType.bypass,
                replica_groups=g8,
                ins=[a2a_in.opt()],
                outs=[a2a_out.opt()],
            )
            for _ in range(20):
                warm_unit(8)
            warm_release()

            attF = []
            for i in range(NCORES):
                t = attF_pool.tile([128, R], bf16, name=f"attF{i}", tag="attF")
                dma_round[i % 3].dma_start(out=t, in_=a2a_out[128 * i : 128 * (i + 1), :])
                attF.append(t)

            # 4-deep PSUM ring across the three pools (all free post-attention)
            op_pools = [ps_mm, ps_att, ps_sc, ps_mm]
            gi = 0
            for m in range(R // 128):
                for n in range(2):
                    ops = op_pools[gi % 4].tile(
                        [128, 512], f32, name=f"ops{m}_{n}",
                        tag=["mm", "att", "sc", "mm"][gi % 4],
                    )
                    gi += 1
                    for k in range(NK):
                        nc.tensor.matmul(
                            out=ops[:],
                            lhsT=attF[k][:, 128 * m : 128 * (m + 1)],
                            rhs=wo_sb[k][:, 512 * n : 512 * (n + 1)],
                            start=(k == 0),
                            stop=(k == NK - 1),
                        )
                    oev = outp.tile([128, 512], f32, tag="oev")
                    nc.vector.tensor_add(
                        out=oev, in0=ops[:], in1=bo_bc[:, 512 * n : 512 * (n + 1)]
                    )
                    nc.sync.dma_start(
                        out=out.ap()[
                            128 * m : 128 * (m + 1), 512 * n : 512 * (n + 1)
                        ],
                        in_=oev,
                    )
    nc.finalize()
    return nc


def get_nc(use_f32r=True):
    key = use_f32r
    if key not in _COMPILED:
        _COMPILED[key] = build_nc(use_f32r)
    return _COMPILED[key]


def make_in_maps(x, Wq, bq, Wk, bk, Wv, bv, Wo, bo):
    bf = ml_dtypes.bfloat16
    x = np.asarray(x, np.float32).reshape(B * S, D)
    xt = x.T.astype(bf)  # [1024, 4096]
    # pre-tile to [32, 128, 1024]: index = cp*8 + k, each tile contiguous
    xT_bf = np.ascontiguousarray(
        np.stack(
            [
                xt[128 * k : 128 * (k + 1), 1024 * cp : 1024 * (cp + 1)]
                for cp in range(4)
                for k in range(8)
            ]
        )
    )
    Wo_bf = np.ascontiguousarray(np.asarray(Wo, np.float32).astype(bf))
    bo_col = np.asarray(bo, np.float32).reshape(D, 1)
    Wq = np.asarray(Wq, np.float32)
    Wk = np.asarray(Wk, np.float32)
    Wv = np.asarray(Wv, np.float32)
    bq = np.asarray(bq, np.float32)
    bk = np.asarray(bk, np.float32)
    bv = np.asarray(bv, np.float32)
    in_maps = []
    for c in range(NCORES):
        sl = slice(F * c, F * (c + 1))
        in_maps.append(
            {
                "xT": xT_bf,
                "Wo": Wo_bf,
                "bo": bo_col,
                "Wq": np.ascontiguousarray(Wq[:, sl].astype(bf)),
                "Wk": np.ascontiguousarray(Wk[:, sl].astype(bf)),
                "Wv": np.ascontiguousarray(Wv[:, sl].astype(bf)),
                "bq": np.ascontiguousarray(bq[sl].reshape(F, 1)),
                "bk": np.ascontiguousarray(bk[sl].reshape(F, 1)),
                "bv": np.ascontiguousarray(bv[sl].reshape(F, 1)),
            }
        )
    return in_maps


def gather_out(results):
    outs = [np.asarray(results[c]["out"], np.float32) for c in range(NCORES)]
    return np.concatenate(outs, axis=0).reshape(B, S, D)


def kernel(x, Wq, bq, Wk, bk, Wv, bv, Wo, bo, _use_f32r=True):
    in_maps = make_in_maps(x, Wq, bq, Wk, bk, Wv, bv, Wo, bo)
    nc = get_nc(_use_f32r)
    res = run_bass_kernel_spmd(nc, in_maps, list(range(NCORES)))
    return gather_out(res.results)
